# revision 31
# baseline (speedup 1.0000x reference)
"""CrossWindowAttention Trainium2 kernel (8 NeuronCores, data-parallel over B).

Layout strategy (per core, 32 batches):
  x^T (host-transposed, ones-row augmented)  ->  Q^T/K^T [192,344] fp32r,
  V [344,192] bf16.  A^T_h = K_h @ Q_h^T (+ identity-matmul folds of
  mask^T and rpb^T in PSUM, bf16).  E = exp(A^T) on ACT (no max-subtract:
  logits bounded).  O^T_h = V_h^T @ E_h and s_h = ones^T @ E_h accumulated
  over key chunks.  O^T /= s via reciprocal + sbuf->sbuf DMA broadcast +
  fused DVE multiply.  Final proj consumes O^T as lhsT so the output lands
  in natural [n, c] layout.
"""

import sys

if "/opt/trn_rl_repo" not in sys.path:
    sys.path.insert(0, "/opt/trn_rl_repo")

import numpy as np
import ml_dtypes
from contextlib import ExitStack

import concourse.bass as bass
import concourse.tile as tile
from concourse import bacc, mybir
from concourse.bass_utils import run_bass_kernel_spmd

dt = mybir.dt

B = 256
N = 343          # tokens per window
NP = 344         # padded token dim (fp32r needs even free sizes)
C = 192
H = 6
D = 32
NW = 64
NCORES = 8
BL = B // NCORES
CA = 194         # c + ones row + zero pad row (even contraction for fp32r)
SCALE = D ** -0.5

F32R = dt.float32r
BF16 = dt.bfloat16
F32 = dt.float32

KCH = [(0, 128), (128, 66)]            # contraction chunks of augmented c
MCH = [(0, 96), (96, 96)]              # c_out chunks (head slices at base 0/32/64)
TCH = [(0, 128), (128, 128), (256, 88)]  # token chunks (row-padded to 344)


def _patch_act_tables():
    """Force one ACT table set that covers both Exp and Ln.

    The default per-activation set selection alternates between
    `exp_and_others` (for Exp) and a ln-set (for Ln), inserting a ~9.5us
    ACT_TABLE_LOAD + pipeline drain per batch.  Emptying every other set
    (indices preserved) makes the placement pass pick the shared
    `natural_log_exp_and_others` set once.
    """
    import concourse.bacc as bacc_mod
    if getattr(bacc_mod, "_act_tables_patched", False):
        return
    real = bacc_mod.get_activation_tables

    def patched(arch):
        t = real(arch)
        return {k: (v if k == "natural_log_exp_and_others" else set())
                for k, v in t.items()}

    bacc_mod.get_activation_tables = patched
    bacc_mod._act_tables_patched = True


def build_program():
    _patch_act_tables()
    nc = bacc.Bacc("TRN2", target_bir_lowering=False, debug=False,
                   num_devices=NCORES)

    d_xq = nc.dram_tensor("xq", [BL, CA, NP], BF16, kind="ExternalInput").ap()
    d_xk = nc.dram_tensor("xk", [BL, CA, NP], BF16, kind="ExternalInput").ap()
    d_mask = nc.dram_tensor("maskT", [BL, NP, NP], BF16, kind="ExternalInput").ap()
    d_rpb = nc.dram_tensor("rpbT", [NP, H * NP], BF16, kind="ExternalInput").ap()
    d_wq = nc.dram_tensor("wq", [CA, C], BF16, kind="ExternalInput").ap()
    d_wk = nc.dram_tensor("wk", [CA, C], BF16, kind="ExternalInput").ap()
    d_wv = nc.dram_tensor("wv", [CA, C], BF16, kind="ExternalInput").ap()
    d_wp = nc.dram_tensor("wp", [CA, C], BF16, kind="ExternalInput").ap()
    d_id = nc.dram_tensor("ident", [128, 128], BF16, kind="ExternalInput").ap()
    d_ones = nc.dram_tensor("onesc", [128, 2], BF16, kind="ExternalInput").ap()
    d_onesb = nc.dram_tensor("onesb", [1, 32], BF16, kind="ExternalInput").ap()
    d_tail = nc.dram_tensor("tail", [2, NP], BF16, kind="ExternalInput").ap()
    d_out = nc.dram_tensor("out", [BL, N, C], F32, kind="ExternalOutput").ap()

    with tile.TileContext(nc) as tc, ExitStack() as ctx:
        cpool = ctx.enter_context(tc.tile_pool(name="const", bufs=1))
        xpool = ctx.enter_context(tc.tile_pool(name="x", bufs=3))
        qkpool = ctx.enter_context(tc.tile_pool(name="qk", bufs=3))
        vpool = ctx.enter_context(tc.tile_pool(name="v", bufs=3))
        mpool = ctx.enter_context(tc.tile_pool(name="maskt", bufs=3))
        epool = ctx.enter_context(tc.tile_pool(name="e", bufs=6))
        evpool = ctx.enter_context(tc.tile_pool(name="ev", bufs=3))
        opool = ctx.enter_context(tc.tile_pool(name="o", bufs=3))
        rpool = ctx.enter_context(tc.tile_pool(name="r", bufs=3))
        fpool = ctx.enter_context(tc.tile_pool(name="fin", bufs=3))
        ps_a = ctx.enter_context(tc.tile_pool(name="ps_a", bufs=1, space="PSUM"))
        ps_pv = ctx.enter_context(tc.tile_pool(name="ps_pv", bufs=1, space="PSUM"))
        ps_s = ctx.enter_context(tc.tile_pool(name="ps_s", bufs=1, space="PSUM"))
        ps_m = ctx.enter_context(tc.tile_pool(name="ps_m", bufs=2, space="PSUM"))

        # ---- resident constants ----
        wq_t, wk_t, wv_t, wp_t = [], [], [], []
        for ki, (ko, kn) in enumerate(KCH):
            for lst, src, dty in ((wq_t, d_wq, BF16), (wk_t, d_wk, BF16),
                                  (wv_t, d_wv, BF16), (wp_t, d_wp, BF16)):
                t = cpool.tile([kn, C], dty, tag=f"w{len(lst)}_{ki}_{id(lst) % 97}")
                nc.sync.dma_start(t[:], src[ko:ko + kn, :])
                lst.append(t)
        id_t = cpool.tile([128, 128], BF16, tag="ident")
        nc.sync.dma_start(id_t[:], d_id[:, :])
        ones_t = cpool.tile([128, 2], BF16, tag="onesc")
        nc.sync.dma_start(ones_t[:], d_ones[:, :])
        onesb_t = cpool.tile([1, 32], BF16, tag="onesb")
        nc.sync.dma_start(onesb_t[:], d_onesb[:, :])
        tail_t = cpool.tile([2, NP], BF16, tag="tail")
        nc.sync.dma_start(tail_t[:], d_tail[:, :])
        rpb_t = []
        for ti, (to, tn) in enumerate(TCH):
            t = cpool.tile([tn, H * NP], BF16, tag=f"rpb{ti}")
            nc.sync.dma_start(t[:], d_rpb[to:to + tn, :])
            rpb_t.append(t)

        def emit_head(b, tail_fn=None):
            # ---- load x^T ----
            xq_t, xk_t = [], []
            for ki, (ko, kn) in enumerate(KCH):
                for lst, srcd, nm in ((xq_t, d_xq, "xq"), (xk_t, d_xk, "xk")):
                    t = xpool.tile([kn, NP], BF16, tag=f"{nm}{ki}")
                    nc.sync.dma_start(t[:], srcd[b, ko:ko + kn, :])
                    lst.append(t)
            mask_t = []
            for ti, (to, tn) in enumerate(TCH):
                t = mpool.tile([tn, NP], BF16, tag=f"mask{ti}")
                nc.scalar.dma_start(t[:], d_mask[b, to:to + tn, :])
                mask_t.append(t)

            # ---- Q^T, K^T projections (fp32r) ----
            qt, kt = [], []
            for w_t, x_t, dest, nm in ((wq_t, xq_t, None, "q"), (wk_t, xk_t, None, "k")):
                dest = qt if nm == "q" else kt
                for mi, (mo, mn) in enumerate(MCH):
                    ps = ps_m.tile([128, 512], F32, tag="mm")
                    for ki in range(len(KCH)):
                        nc.tensor.matmul(ps[0:mn, 0:NP],
                                         w_t[ki][:, mo:mo + mn],
                                         x_t[ki][:],
                                         start=(ki == 0), stop=(ki == len(KCH) - 1))
                    sb = qkpool.tile([96, NP], BF16, tag=f"{nm}{mi}")
                    nc.vector.tensor_copy(sb[:], ps[0:mn, 0:NP])
                    dest.append(sb)

            # ---- V projection (bf16, natural layout) ----
            v_t = []
            for ti, (to, tn) in enumerate(TCH):
                ps = ps_m.tile([128, 512], F32, tag="mm")
                for ki in range(len(KCH)):
                    nc.tensor.matmul(ps[0:tn, 0:C],
                                     xk_t[ki][:, to:to + tn],
                                     wv_t[ki][:],
                                     start=(ki == 0), stop=(ki == len(KCH) - 1))
                sb = vpool.tile([128, C], BF16, tag=f"v{ti}")
                nc.vector.tensor_copy(sb[0:tn, :], ps[0:tn, 0:C])
                v_t.append(sb)

            # ---- attention ----
            pv_ps = ps_pv.tile([128, 2, 512], F32, tag="pv")
            s_ps = ps_s.tile([128, 1, 512], F32, tag="s")
            for ci, (co_, cn) in enumerate(TCH):
                e_ts = []
                for g in range(2):
                    a_ps = ps_a.tile([128, 3, 512], F32, tag="a")
                    # QK matmuls first (row-tiled mode, concurrent heads)
                    for hh in range(3):
                        h = 3 * g + hh
                        t_i, r_off = (0, 32 * h) if h < 3 else (1, 32 * (h - 3))
                        nc.tensor.matmul(
                            a_ps[0:cn, hh, 0:NP],
                            kt[t_i][r_off:r_off + D, co_:co_ + cn],
                            qt[t_i][r_off:r_off + D, :],
                            start=True, stop=False)
                    # bias folds (128x128 mode)
                    for hh in range(3):
                        h = 3 * g + hh
                        nc.tensor.matmul(
                            a_ps[0:cn, hh, 0:NP],
                            id_t[0:cn, 0:cn],
                            mask_t[ci][:, :],
                            start=False, stop=False)
                        nc.tensor.matmul(
                            a_ps[0:cn, hh, 0:NP],
                            id_t[0:cn, 0:cn],
                            rpb_t[ci][:, h * NP:(h + 1) * NP],
                            start=False, stop=True)
                    e_t = epool.tile([128, 3 * NP], BF16, tag="e")
                    nc.scalar.activation(
                        e_t[0:cn, :].rearrange("p (r n) -> p r n", r=3),
                        a_ps[0:cn, 0:3, 0:NP],
                        mybir.ActivationFunctionType.Exp)
                    e_ts.append(e_t)

                if ci == 0 and tail_fn is not None:
                    # inject previous batch's tail here: its ACT ops land
                    # early in the ACT FIFO, its proj matmuls after this
                    # chunk's in the PE FIFO.
                    tail_fn()

                for g in range(2):
                    for hh in range(3):
                        h = 3 * g + hh
                        bank, base = (0, 32 * h) if h < 4 else (1, 32 * (h - 4))
                        nc.tensor.matmul(
                            pv_ps[base:base + D, bank, 0:NP],
                            v_t[ci][0:cn, 32 * h:32 * h + D],
                            e_ts[g][0:cn, hh * NP:(hh + 1) * NP],
                            start=(ci == 0), stop=(ci == len(TCH) - 1),
                            tile_position=(0, base))
                        # s rows: heads 0-3 in s bank at rows 32h;
                        # heads 4,5 tucked into pv bank1 rows 64/96.
                        if h < 4:
                            s_out = s_ps[32 * h:32 * h + 1, 0, 0:NP]
                            s_tp = (0, 32 * h)
                        else:
                            s_out = pv_ps[32 * (h - 2):32 * (h - 2) + 1, 1, 0:NP]
                            s_tp = (0, 32 * (h - 2))
                        nc.tensor.matmul(
                            s_out,
                            ones_t[0:cn, 0:1],
                            e_ts[g][0:cn, hh * NP:(hh + 1) * NP],
                            start=(ci == 0), stop=(ci == len(TCH) - 1),
                            tile_position=s_tp)

            return pv_ps, s_ps

        def emit_evac(handles):
            pv_ps, s_ps = handles
            # ---- evacuate PSUM -> SBUF so next batch can reuse pv/s banks ----
            # ev_so layout: cols 0:NP = s rows h0-3 (junk elsewhere);
            # cols NP:2NP = pv bank1 (h4/h5 O rows 0-63, s4/s5 at rows 64/96)
            o1_sb = evpool.tile([128, NP], F32, tag="o1")
            so_sb = evpool.tile([128, 2 * NP], F32, tag="so")
            nc.vector.tensor_copy(o1_sb[:], pv_ps[0:128, 0, 0:NP])
            nc.vector.tensor_copy(so_sb[:, NP:2 * NP], pv_ps[0:128, 1, 0:NP])
            nc.scalar.copy(so_sb[:, 0:NP], s_ps[0:128, 0, 0:NP])
            return o1_sb, so_sb

        def emit_tail_act(handles):
            o1_sb, so_sb = handles
            # ---- r = exp(-ln(s)) on ACT (shared table set with the E exp;
            # DVE reciprocal is an 8-cycle/elem iterative op) ----
            lnr = rpool.tile([128, 2 * NP], F32, tag="lnr")
            nc.scalar.activation(lnr[:], so_sb[:],
                                 mybir.ActivationFunctionType.Ln)
            r_sb = rpool.tile([128, 2 * NP], BF16, tag="r")
            nc.scalar.activation(r_sb[:], lnr[:],
                                 mybir.ActivationFunctionType.Exp, scale=-1.0)

            # ---- gather the 6 r rows into one partition (2 tiny DMAs),
            # then broadcast via K=1 outer-product matmuls into PSUM ----
            rmv = rpool.tile([1, 6 * NP], BF16, tag="rmv")
            rowlen = r_sb[:].tensor.shape[-1]
            base_off = r_sb[:].offset
            src = bass.AP(tensor=r_sb[:].tensor, offset=base_off,
                          ap=[[32 * rowlen, 4], [1, NP]])
            nc.scalar.dma_start(rmv[0:1, 0:4 * NP], src)
            src = bass.AP(tensor=r_sb[:].tensor,
                          offset=base_off + 64 * rowlen + NP,
                          ap=[[32 * rowlen, 2], [1, NP]])
            nc.scalar.dma_start(rmv[0:1, 4 * NP:6 * NP], src)

            rb1 = ps_m.tile([128, 512], F32, tag="mm")
            rb2 = ps_m.tile([128, 512], F32, tag="mm")
            for h in range(4):
                nc.tensor.matmul(rb1[32 * h:32 * h + 32, 0:NP],
                                 onesb_t[0:1, :],
                                 rmv[0:1, h * NP:(h + 1) * NP],
                                 start=True, stop=True, tile_position=(0, 32 * h))
            for h in range(2):
                nc.tensor.matmul(rb2[32 * h:32 * h + 32, 0:NP],
                                 onesb_t[0:1, :],
                                 rmv[0:1, (4 + h) * NP:(5 + h) * NP],
                                 start=True, stop=True, tile_position=(0, 32 * h))
            return rb1, rb2

        def emit_tail_pe(b, handles, rbs):
            o1_sb, so_sb = handles
            rb1, rb2 = rbs
            # ---- normalize O^T -> head-stacked Ostack (fp32r) ----
            ost1 = opool.tile([128, NP], BF16, tag="ost1")
            ost2 = opool.tile([66, NP], BF16, tag="ost2")
            nc.vector.scalar_tensor_tensor(
                ost1[:], o1_sb[:], 1.0, rb1[0:128, 0:NP],
                mybir.AluOpType.mult, mybir.AluOpType.mult)
            nc.vector.scalar_tensor_tensor(
                ost2[0:64, :], so_sb[0:64, NP:2 * NP], 1.0, rb2[0:64, 0:NP],
                mybir.AluOpType.mult, mybir.AluOpType.mult)
            if b < 3:  # pool has 3 slots; the ones/zero tail rows persist
                nc.sync.dma_start(ost2[64:66, :], tail_t[:])

            # ---- output projection (fp32r) + store ----
            ost = [ost1, ost2]
            for ti, (to, tn) in enumerate(TCH):
                ps = ps_m.tile([128, 512], F32, tag="mm")
                for ki in range(2):
                    nc.tensor.matmul(ps[0:tn, 0:C],
                                     ost[ki][:, to:to + tn],
                                     wp_t[ki][:],
                                     start=(ki == 0), stop=(ki == 1))
                f_sb = fpool.tile([128, C], F32, tag="f")
                nc.vector.tensor_copy(f_sb[0:tn, :], ps[0:tn, 0:C])
                rows = min(tn, N - to)
                nc.scalar.dma_start(d_out[b, to:to + rows, :], f_sb[0:rows, :])

        # software pipeline: batch b's tail ACT-chain is injected inside
        # batch b+1's head (after chunk-0 exps, so ln/exp land early in the
        # ACT FIFO); the stt+proj tail runs after the full head; evacuation
        # last so the DVE FIFO has stt(b-1) before evac(b).
        prev = None
        rb_box = {}
        for b in range(BL):
            if prev is not None:
                pe = prev
                tail_fn = lambda pe=pe: rb_box.__setitem__(0, emit_tail_act(pe))
            else:
                tail_fn = None
            ps_handles = emit_head(b, tail_fn)
            if prev is not None:
                emit_tail_pe(b - 1, prev, rb_box[0])
            prev = emit_evac(ps_handles)
        rbs = emit_tail_act(prev)
        emit_tail_pe(BL - 1, prev, rbs)

    nc.compile()
    return nc


_NC_CACHE = None


def _get_program():
    global _NC_CACHE
    if _NC_CACHE is None:
        _NC_CACHE = build_program()
    return _NC_CACHE


def _prep_inputs(x_q, x_kv, mask, q_w, q_b, kv_w, kv_b, proj_w, proj_b,
                 rpb_table, rpi):
    bf16 = ml_dtypes.bfloat16
    f32 = np.float32

    def aug_w(w, bias, scale=1.0):
        m = np.zeros((CA, C), f32)
        m[:C] = np.asarray(w, f32).T
        m[C] = np.asarray(bias, f32)
        return np.ascontiguousarray(m * scale)

    wq = aug_w(q_w, q_b, SCALE).astype(bf16)
    wk = aug_w(kv_w[:C], kv_b[:C]).astype(bf16)
    wv = aug_w(kv_w[C:], kv_b[C:]).astype(bf16)
    wp = aug_w(proj_w, proj_b).astype(bf16)

    def xT_aug(x):
        out = np.zeros((B, CA, NP), f32)
        out[:, :C, :N] = np.asarray(x, f32).transpose(0, 2, 1)
        out[:, C, :N] = 1.0
        return out

    xqT = xT_aug(x_q).astype(bf16)
    xkT = xT_aug(x_kv).astype(bf16)

    maskT = np.full((NW, NP, NP), -100.0, f32)
    maskT[:, :N, :N] = np.asarray(mask, f32).transpose(0, 2, 1)
    maskT = maskT.astype(bf16)

    g = np.asarray(rpb_table, f32)[np.asarray(rpi)]        # [i, j, H]
    rpbT = np.zeros((NP, H, NP), f32)
    rpbT[:N, :, :N] = g.transpose(1, 2, 0)
    rpbT = rpbT.reshape(NP, H * NP).astype(bf16)

    ident = np.eye(128, dtype=f32).astype(bf16)
    onesc = np.ones((128, 2), f32).astype(bf16)
    tail = np.zeros((2, NP), f32)
    tail[0, :N] = 1.0
    tail = tail.astype(bf16)

    in_maps = []
    for cidx in range(NCORES):
        sl = slice(cidx * BL, (cidx + 1) * BL)
        w0 = (cidx * BL) % NW
        in_maps.append({
            "xq": xqT[sl], "xk": xkT[sl],
            "maskT": maskT[w0:w0 + BL], "rpbT": rpbT,
            "wq": wq, "wk": wk, "wv": wv, "wp": wp,
            "ident": ident, "onesc": onesc, "tail": tail,
            "onesb": np.ones((1, 32), np.float32).astype(bf16),
        })
    return in_maps


def kernel(x_q, x_kv, mask, q_w, q_b, kv_w, kv_b, proj_w, proj_b,
           rpb_table, rpi):
    nc = _get_program()
    in_maps = _prep_inputs(x_q, x_kv, mask, q_w, q_b, kv_w, kv_b,
                           proj_w, proj_b, rpb_table, rpi)
    res = run_bass_kernel_spmd(nc, in_maps, core_ids=list(range(NCORES)),
                               trace=False)
    out = np.concatenate([res.results[i]["out"] for i in range(NCORES)], 0)
    return np.ascontiguousarray(out.astype(np.float32))


def run_traced(inputs, trace=True):
    """test-harness entry: returns (output, exec_time_ns, results_obj)."""
    nc = _get_program()
    in_maps = _prep_inputs(**inputs)
    res = run_bass_kernel_spmd(nc, in_maps, core_ids=list(range(NCORES)),
                               trace=trace)
    out = np.concatenate([res.results[i]["out"] for i in range(NCORES)], 0)
    return np.ascontiguousarray(out.astype(np.float32)), res.exec_time_ns, res


# revision 32
# speedup vs baseline: 1.0134x; 1.0134x over previous
"""CrossWindowAttention Trainium2 kernel (8 NeuronCores, data-parallel over B).

Layout strategy (per core, 32 batches):
  x^T (host-transposed, ones-row augmented)  ->  Q^T/K^T [192,344] fp32r,
  V [344,192] bf16.  A^T_h = K_h @ Q_h^T (+ identity-matmul folds of
  mask^T and rpb^T in PSUM, bf16).  E = exp(A^T) on ACT (no max-subtract:
  logits bounded).  O^T_h = V_h^T @ E_h and s_h = ones^T @ E_h accumulated
  over key chunks.  O^T /= s via reciprocal + sbuf->sbuf DMA broadcast +
  fused DVE multiply.  Final proj consumes O^T as lhsT so the output lands
  in natural [n, c] layout.
"""

import sys

if "/opt/trn_rl_repo" not in sys.path:
    sys.path.insert(0, "/opt/trn_rl_repo")

import numpy as np
import ml_dtypes
from contextlib import ExitStack

import concourse.bass as bass
import concourse.tile as tile
from concourse import bacc, mybir
from concourse.bass_utils import run_bass_kernel_spmd

dt = mybir.dt

B = 256
N = 343          # tokens per window
NP = 344         # padded token dim (fp32r needs even free sizes)
C = 192
H = 6
D = 32
NW = 64
NCORES = 8
BL = B // NCORES
CA = 194         # c + ones row + zero pad row (even contraction for fp32r)
SCALE = D ** -0.5

F32R = dt.float32r
BF16 = dt.bfloat16
F32 = dt.float32

KCH = [(0, 128), (128, 66)]            # contraction chunks of augmented c
MCH = [(0, 96), (96, 96)]              # c_out chunks (head slices at base 0/32/64)
TCH = [(0, 128), (128, 128), (256, 88)]  # token chunks (row-padded to 344)


def _patch_act_tables():
    """Force one ACT table set that covers both Exp and Ln.

    The default per-activation set selection alternates between
    `exp_and_others` (for Exp) and a ln-set (for Ln), inserting a ~9.5us
    ACT_TABLE_LOAD + pipeline drain per batch.  Emptying every other set
    (indices preserved) makes the placement pass pick the shared
    `natural_log_exp_and_others` set once.
    """
    import concourse.bacc as bacc_mod
    if getattr(bacc_mod, "_act_tables_patched", False):
        return
    real = bacc_mod.get_activation_tables

    def patched(arch):
        t = real(arch)
        return {k: (v if k == "natural_log_exp_and_others" else set())
                for k, v in t.items()}

    bacc_mod.get_activation_tables = patched
    bacc_mod._act_tables_patched = True


def build_program():
    _patch_act_tables()
    nc = bacc.Bacc("TRN2", target_bir_lowering=False, debug=False,
                   num_devices=NCORES)

    d_xq = nc.dram_tensor("xq", [BL, CA, NP], BF16, kind="ExternalInput").ap()
    d_xk = nc.dram_tensor("xk", [BL, CA, NP], BF16, kind="ExternalInput").ap()
    d_mask = nc.dram_tensor("maskT", [BL, NP, NP], BF16, kind="ExternalInput").ap()
    d_rpb = nc.dram_tensor("rpbT", [NP, H * NP], BF16, kind="ExternalInput").ap()
    d_wq = nc.dram_tensor("wq", [CA, C], BF16, kind="ExternalInput").ap()
    d_wk = nc.dram_tensor("wk", [CA, C], BF16, kind="ExternalInput").ap()
    d_wv = nc.dram_tensor("wv", [CA, C], BF16, kind="ExternalInput").ap()
    d_wp = nc.dram_tensor("wp", [CA, C], BF16, kind="ExternalInput").ap()
    d_id = nc.dram_tensor("ident", [128, 128], BF16, kind="ExternalInput").ap()
    d_ones = nc.dram_tensor("onesc", [128, 2], BF16, kind="ExternalInput").ap()
    d_onesb = nc.dram_tensor("onesb", [1, 32], BF16, kind="ExternalInput").ap()
    d_tail = nc.dram_tensor("tail", [2, NP], BF16, kind="ExternalInput").ap()
    d_out = nc.dram_tensor("out", [BL, N, C], F32, kind="ExternalOutput").ap()

    with tile.TileContext(nc) as tc, ExitStack() as ctx:
        cpool = ctx.enter_context(tc.tile_pool(name="const", bufs=1))
        xpool = ctx.enter_context(tc.tile_pool(name="x", bufs=2))
        qkpool = ctx.enter_context(tc.tile_pool(name="qk", bufs=2))
        vpool = ctx.enter_context(tc.tile_pool(name="v", bufs=2))
        mpool = ctx.enter_context(tc.tile_pool(name="maskt", bufs=2))
        epool = ctx.enter_context(tc.tile_pool(name="e", bufs=6))
        evpool = ctx.enter_context(tc.tile_pool(name="ev", bufs=3))
        opool = ctx.enter_context(tc.tile_pool(name="o", bufs=3))
        rpool = ctx.enter_context(tc.tile_pool(name="r", bufs=3))
        fpool = ctx.enter_context(tc.tile_pool(name="fin", bufs=3))
        ps_a = ctx.enter_context(tc.tile_pool(name="ps_a", bufs=1, space="PSUM"))
        ps_pv = ctx.enter_context(tc.tile_pool(name="ps_pv", bufs=1, space="PSUM"))
        ps_s = ctx.enter_context(tc.tile_pool(name="ps_s", bufs=1, space="PSUM"))
        ps_m = ctx.enter_context(tc.tile_pool(name="ps_m", bufs=2, space="PSUM"))

        # ---- resident constants ----
        wq_t, wk_t, wv_t, wp_t = [], [], [], []
        for ki, (ko, kn) in enumerate(KCH):
            for lst, src, dty in ((wq_t, d_wq, BF16), (wk_t, d_wk, BF16),
                                  (wv_t, d_wv, BF16), (wp_t, d_wp, BF16)):
                t = cpool.tile([kn, C], dty, tag=f"w{len(lst)}_{ki}_{id(lst) % 97}")
                nc.sync.dma_start(t[:], src[ko:ko + kn, :])
                lst.append(t)
        id_t = cpool.tile([128, 128], BF16, tag="ident")
        nc.sync.dma_start(id_t[:], d_id[:, :])
        ones_t = cpool.tile([128, 2], BF16, tag="onesc")
        nc.sync.dma_start(ones_t[:], d_ones[:, :])
        onesb_t = cpool.tile([1, 32], BF16, tag="onesb")
        nc.sync.dma_start(onesb_t[:], d_onesb[:, :])
        tail_t = cpool.tile([2, NP], BF16, tag="tail")
        nc.sync.dma_start(tail_t[:], d_tail[:, :])
        rpb_t = []
        for ti, (to, tn) in enumerate(TCH):
            t = cpool.tile([tn, H * NP], BF16, tag=f"rpb{ti}")
            nc.sync.dma_start(t[:], d_rpb[to:to + tn, :])
            rpb_t.append(t)

        def emit_head(b, tail_fn=None):
            # ---- load x^T ----
            xq_t, xk_t = [], []
            for ki, (ko, kn) in enumerate(KCH):
                for lst, srcd, nm in ((xq_t, d_xq, "xq"), (xk_t, d_xk, "xk")):
                    t = xpool.tile([kn, NP], BF16, tag=f"{nm}{ki}")
                    nc.sync.dma_start(t[:], srcd[b, ko:ko + kn, :])
                    lst.append(t)
            mask_t = []
            for ti, (to, tn) in enumerate(TCH):
                t = mpool.tile([tn, NP], BF16, tag=f"mask{ti}")
                nc.scalar.dma_start(t[:], d_mask[b, to:to + tn, :])
                mask_t.append(t)

            # ---- Q^T, K^T projections (fp32r) ----
            qt, kt = [], []
            for w_t, x_t, dest, nm in ((wq_t, xq_t, None, "q"), (wk_t, xk_t, None, "k")):
                dest = qt if nm == "q" else kt
                for mi, (mo, mn) in enumerate(MCH):
                    ps = ps_m.tile([128, 512], F32, tag="mm")
                    for ki in range(len(KCH)):
                        nc.tensor.matmul(ps[0:mn, 0:NP],
                                         w_t[ki][:, mo:mo + mn],
                                         x_t[ki][:],
                                         start=(ki == 0), stop=(ki == len(KCH) - 1))
                    sb = qkpool.tile([96, NP], BF16, tag=f"{nm}{mi}")
                    nc.vector.tensor_copy(sb[:], ps[0:mn, 0:NP])
                    dest.append(sb)

            # ---- V projection (bf16, natural layout) ----
            v_t = []
            for ti, (to, tn) in enumerate(TCH):
                ps = ps_m.tile([128, 512], F32, tag="mm")
                for ki in range(len(KCH)):
                    nc.tensor.matmul(ps[0:tn, 0:C],
                                     xk_t[ki][:, to:to + tn],
                                     wv_t[ki][:],
                                     start=(ki == 0), stop=(ki == len(KCH) - 1))
                sb = vpool.tile([128, C], BF16, tag=f"v{ti}")
                nc.vector.tensor_copy(sb[0:tn, :], ps[0:tn, 0:C])
                v_t.append(sb)

            # ---- attention ----
            pv_ps = ps_pv.tile([128, 2, 512], F32, tag="pv")
            s_ps = ps_s.tile([128, 1, 512], F32, tag="s")
            for ci, (co_, cn) in enumerate(TCH):
                e_ts = []
                for g in range(2):
                    a_ps = ps_a.tile([128, 3, 512], F32, tag="a")
                    # QK matmuls first (row-tiled mode, concurrent heads)
                    for hh in range(3):
                        h = 3 * g + hh
                        t_i, r_off = (0, 32 * h) if h < 3 else (1, 32 * (h - 3))
                        nc.tensor.matmul(
                            a_ps[0:cn, hh, 0:NP],
                            kt[t_i][r_off:r_off + D, co_:co_ + cn],
                            qt[t_i][r_off:r_off + D, :],
                            start=True, stop=False)
                    # bias folds (128x128 mode)
                    for hh in range(3):
                        h = 3 * g + hh
                        nc.tensor.matmul(
                            a_ps[0:cn, hh, 0:NP],
                            id_t[0:cn, 0:cn],
                            mask_t[ci][:, :],
                            start=False, stop=False)
                        nc.tensor.matmul(
                            a_ps[0:cn, hh, 0:NP],
                            id_t[0:cn, 0:cn],
                            rpb_t[ci][:, h * NP:(h + 1) * NP],
                            start=False, stop=True)
                    e_t = epool.tile([128, 3 * NP], BF16, tag="e")
                    nc.scalar.activation(
                        e_t[0:cn, :].rearrange("p (r n) -> p r n", r=3),
                        a_ps[0:cn, 0:3, 0:NP],
                        mybir.ActivationFunctionType.Exp)
                    e_ts.append(e_t)

                if ci == 0 and tail_fn is not None:
                    # inject previous batch's tail here: its ACT ops land
                    # early in the ACT FIFO, its proj matmuls after this
                    # chunk's in the PE FIFO.
                    tail_fn()

                for g in range(2):
                    for hh in range(3):
                        h = 3 * g + hh
                        bank, base = (0, 32 * h) if h < 4 else (1, 32 * (h - 4))
                        nc.tensor.matmul(
                            pv_ps[base:base + D, bank, 0:NP],
                            v_t[ci][0:cn, 32 * h:32 * h + D],
                            e_ts[g][0:cn, hh * NP:(hh + 1) * NP],
                            start=(ci == 0), stop=(ci == len(TCH) - 1),
                            tile_position=(0, base))
                        # s rows: heads 0-3 in s bank at rows 32h;
                        # heads 4,5 tucked into pv bank1 rows 64/96.
                        if h < 4:
                            s_out = s_ps[32 * h:32 * h + 1, 0, 0:NP]
                            s_tp = (0, 32 * h)
                        else:
                            s_out = pv_ps[32 * (h - 2):32 * (h - 2) + 1, 1, 0:NP]
                            s_tp = (0, 32 * (h - 2))
                        nc.tensor.matmul(
                            s_out,
                            ones_t[0:cn, 0:1],
                            e_ts[g][0:cn, hh * NP:(hh + 1) * NP],
                            start=(ci == 0), stop=(ci == len(TCH) - 1),
                            tile_position=s_tp)

            return pv_ps, s_ps

        def emit_evac(handles):
            pv_ps, s_ps = handles
            # ---- evacuate PSUM -> SBUF so next batch can reuse pv/s banks ----
            # ev_so layout: cols 0:NP = s rows h0-3 (junk elsewhere);
            # cols NP:2NP = pv bank1 (h4/h5 O rows 0-63, s4/s5 at rows 64/96)
            o1_sb = evpool.tile([128, NP], F32, tag="o1")
            so_sb = evpool.tile([128, 2 * NP], F32, tag="so")
            nc.vector.tensor_copy(o1_sb[:], pv_ps[0:128, 0, 0:NP])
            nc.vector.tensor_copy(so_sb[:, NP:2 * NP], pv_ps[0:128, 1, 0:NP])
            nc.scalar.copy(so_sb[:, 0:NP], s_ps[0:128, 0, 0:NP])
            return o1_sb, so_sb

        def emit_tail_act(handles):
            o1_sb, so_sb = handles
            # ---- r = exp(-ln(s)) on ACT (shared table set with the E exp;
            # DVE reciprocal is an 8-cycle/elem iterative op) ----
            lnr = rpool.tile([128, 2 * NP], F32, tag="lnr")
            nc.scalar.activation(lnr[:], so_sb[:],
                                 mybir.ActivationFunctionType.Ln)
            r_sb = rpool.tile([128, 2 * NP], BF16, tag="r")
            nc.scalar.activation(r_sb[:], lnr[:],
                                 mybir.ActivationFunctionType.Exp, scale=-1.0)

            # ---- gather the 6 r rows into one partition (2 tiny DMAs),
            # then broadcast via K=1 outer-product matmuls into PSUM ----
            rmv = rpool.tile([1, 6 * NP], BF16, tag="rmv")
            rowlen = r_sb[:].tensor.shape[-1]
            base_off = r_sb[:].offset
            src = bass.AP(tensor=r_sb[:].tensor, offset=base_off,
                          ap=[[32 * rowlen, 4], [1, NP]])
            nc.scalar.dma_start(rmv[0:1, 0:4 * NP], src)
            src = bass.AP(tensor=r_sb[:].tensor,
                          offset=base_off + 64 * rowlen + NP,
                          ap=[[32 * rowlen, 2], [1, NP]])
            nc.scalar.dma_start(rmv[0:1, 4 * NP:6 * NP], src)

            rb1 = ps_m.tile([128, 512], F32, tag="mm")
            rb2 = ps_m.tile([128, 512], F32, tag="mm")
            for h in range(4):
                nc.tensor.matmul(rb1[32 * h:32 * h + 32, 0:NP],
                                 onesb_t[0:1, :],
                                 rmv[0:1, h * NP:(h + 1) * NP],
                                 start=True, stop=True, tile_position=(0, 32 * h))
            for h in range(2):
                nc.tensor.matmul(rb2[32 * h:32 * h + 32, 0:NP],
                                 onesb_t[0:1, :],
                                 rmv[0:1, (4 + h) * NP:(5 + h) * NP],
                                 start=True, stop=True, tile_position=(0, 32 * h))
            return rb1, rb2

        def emit_tail_pe(b, handles, rbs):
            o1_sb, so_sb = handles
            rb1, rb2 = rbs
            # ---- normalize O^T -> head-stacked Ostack (fp32r) ----
            ost1 = opool.tile([128, NP], BF16, tag="ost1")
            ost2 = opool.tile([66, NP], BF16, tag="ost2")
            nc.vector.scalar_tensor_tensor(
                ost1[:], o1_sb[:], 1.0, rb1[0:128, 0:NP],
                mybir.AluOpType.mult, mybir.AluOpType.mult)
            nc.vector.scalar_tensor_tensor(
                ost2[0:64, :], so_sb[0:64, NP:2 * NP], 1.0, rb2[0:64, 0:NP],
                mybir.AluOpType.mult, mybir.AluOpType.mult)
            if b < 3:  # pool has 3 slots; the ones/zero tail rows persist
                nc.sync.dma_start(ost2[64:66, :], tail_t[:])

            # ---- output projection (fp32r) + store ----
            ost = [ost1, ost2]
            for ti, (to, tn) in enumerate(TCH):
                ps = ps_m.tile([128, 512], F32, tag="mm")
                for ki in range(2):
                    nc.tensor.matmul(ps[0:tn, 0:C],
                                     ost[ki][:, to:to + tn],
                                     wp_t[ki][:],
                                     start=(ki == 0), stop=(ki == 1))
                f_sb = fpool.tile([128, C], F32, tag="f")
                nc.vector.tensor_copy(f_sb[0:tn, :], ps[0:tn, 0:C])
                rows = min(tn, N - to)
                nc.scalar.dma_start(d_out[b, to:to + rows, :], f_sb[0:rows, :])

        # software pipeline: batch b's tail ACT-chain is injected inside
        # batch b+1's head (after chunk-0 exps, so ln/exp land early in the
        # ACT FIFO); the stt+proj tail runs after the full head; evacuation
        # last so the DVE FIFO has stt(b-1) before evac(b).
        prev = None
        rb_box = {}
        for b in range(BL):
            if prev is not None:
                pe = prev
                tail_fn = lambda pe=pe: rb_box.__setitem__(0, emit_tail_act(pe))
            else:
                tail_fn = None
            ps_handles = emit_head(b, tail_fn)
            if prev is not None:
                emit_tail_pe(b - 1, prev, rb_box[0])
            prev = emit_evac(ps_handles)
        rbs = emit_tail_act(prev)
        emit_tail_pe(BL - 1, prev, rbs)

    nc.compile()
    return nc


_NC_CACHE = None


def _get_program():
    global _NC_CACHE
    if _NC_CACHE is None:
        _NC_CACHE = build_program()
    return _NC_CACHE


def _prep_inputs(x_q, x_kv, mask, q_w, q_b, kv_w, kv_b, proj_w, proj_b,
                 rpb_table, rpi):
    bf16 = ml_dtypes.bfloat16
    f32 = np.float32

    def aug_w(w, bias, scale=1.0):
        m = np.zeros((CA, C), f32)
        m[:C] = np.asarray(w, f32).T
        m[C] = np.asarray(bias, f32)
        return np.ascontiguousarray(m * scale)

    wq = aug_w(q_w, q_b, SCALE).astype(bf16)
    wk = aug_w(kv_w[:C], kv_b[:C]).astype(bf16)
    wv = aug_w(kv_w[C:], kv_b[C:]).astype(bf16)
    wp = aug_w(proj_w, proj_b).astype(bf16)

    def xT_aug(x):
        out = np.zeros((B, CA, NP), f32)
        out[:, :C, :N] = np.asarray(x, f32).transpose(0, 2, 1)
        out[:, C, :N] = 1.0
        return out

    xqT = xT_aug(x_q).astype(bf16)
    xkT = xT_aug(x_kv).astype(bf16)

    maskT = np.full((NW, NP, NP), -100.0, f32)
    maskT[:, :N, :N] = np.asarray(mask, f32).transpose(0, 2, 1)
    maskT = maskT.astype(bf16)

    g = np.asarray(rpb_table, f32)[np.asarray(rpi)]        # [i, j, H]
    rpbT = np.zeros((NP, H, NP), f32)
    rpbT[:N, :, :N] = g.transpose(1, 2, 0)
    rpbT = rpbT.reshape(NP, H * NP).astype(bf16)

    ident = np.eye(128, dtype=f32).astype(bf16)
    onesc = np.ones((128, 2), f32).astype(bf16)
    tail = np.zeros((2, NP), f32)
    tail[0, :N] = 1.0
    tail = tail.astype(bf16)

    in_maps = []
    for cidx in range(NCORES):
        sl = slice(cidx * BL, (cidx + 1) * BL)
        w0 = (cidx * BL) % NW
        in_maps.append({
            "xq": xqT[sl], "xk": xkT[sl],
            "maskT": maskT[w0:w0 + BL], "rpbT": rpbT,
            "wq": wq, "wk": wk, "wv": wv, "wp": wp,
            "ident": ident, "onesc": onesc, "tail": tail,
            "onesb": np.ones((1, 32), np.float32).astype(bf16),
        })
    return in_maps


def kernel(x_q, x_kv, mask, q_w, q_b, kv_w, kv_b, proj_w, proj_b,
           rpb_table, rpi):
    nc = _get_program()
    in_maps = _prep_inputs(x_q, x_kv, mask, q_w, q_b, kv_w, kv_b,
                           proj_w, proj_b, rpb_table, rpi)
    res = run_bass_kernel_spmd(nc, in_maps, core_ids=list(range(NCORES)),
                               trace=False)
    out = np.concatenate([res.results[i]["out"] for i in range(NCORES)], 0)
    return np.ascontiguousarray(out.astype(np.float32))


def run_traced(inputs, trace=True):
    """test-harness entry: returns (output, exec_time_ns, results_obj)."""
    nc = _get_program()
    in_maps = _prep_inputs(**inputs)
    res = run_bass_kernel_spmd(nc, in_maps, core_ids=list(range(NCORES)),
                               trace=trace)
    out = np.concatenate([res.results[i]["out"] for i in range(NCORES)], 0)
    return np.ascontiguousarray(out.astype(np.float32)), res.exec_time_ns, res


# revision 33
# speedup vs baseline: 1.0773x; 1.0631x over previous
"""CrossWindowAttention Trainium2 kernel (8 NeuronCores, data-parallel over B).

Layout strategy (per core, 32 batches):
  x^T (host-transposed, ones-row augmented)  ->  Q^T/K^T [192,344] fp32r,
  V [344,192] bf16.  A^T_h = K_h @ Q_h^T (+ identity-matmul folds of
  mask^T and rpb^T in PSUM, bf16).  E = exp(A^T) on ACT (no max-subtract:
  logits bounded).  O^T_h = V_h^T @ E_h and s_h = ones^T @ E_h accumulated
  over key chunks.  O^T /= s via reciprocal + sbuf->sbuf DMA broadcast +
  fused DVE multiply.  Final proj consumes O^T as lhsT so the output lands
  in natural [n, c] layout.
"""

import sys

if "/opt/trn_rl_repo" not in sys.path:
    sys.path.insert(0, "/opt/trn_rl_repo")

import numpy as np
import ml_dtypes
from contextlib import ExitStack

import concourse.bass as bass
import concourse.tile as tile
from concourse import bacc, mybir
from concourse.bass_utils import run_bass_kernel_spmd

dt = mybir.dt

B = 256
N = 343          # tokens per window
NP = 344         # padded token dim (fp32r needs even free sizes)
C = 192
H = 6
D = 32
NW = 64
NCORES = 8
BL = B // NCORES
CA = 194         # c + ones row + zero pad row (even contraction for fp32r)
SCALE = D ** -0.5

F32R = dt.float32r
BF16 = dt.bfloat16
F32 = dt.float32

KCH = [(0, 128), (128, 66)]            # contraction chunks of augmented c
MCH = [(0, 96), (96, 96)]              # c_out chunks (head slices at base 0/32/64)
TCH = [(0, 128), (128, 128), (256, 88)]  # token chunks (row-padded to 344)


def _patch_act_tables():
    """Force one ACT table set that covers both Exp and Ln.

    The default per-activation set selection alternates between
    `exp_and_others` (for Exp) and a ln-set (for Ln), inserting a ~9.5us
    ACT_TABLE_LOAD + pipeline drain per batch.  Emptying every other set
    (indices preserved) makes the placement pass pick the shared
    `natural_log_exp_and_others` set once.
    """
    import concourse.bacc as bacc_mod
    if getattr(bacc_mod, "_act_tables_patched", False):
        return
    real = bacc_mod.get_activation_tables

    def patched(arch):
        t = real(arch)
        return {k: (v if k == "natural_log_exp_and_others" else set())
                for k, v in t.items()}

    bacc_mod.get_activation_tables = patched
    bacc_mod._act_tables_patched = True


def build_program():
    _patch_act_tables()
    nc = bacc.Bacc("TRN2", target_bir_lowering=False, debug=False,
                   num_devices=NCORES)

    d_xq = nc.dram_tensor("xq", [BL, CA, NP], BF16, kind="ExternalInput").ap()
    d_xk = nc.dram_tensor("xk", [BL, CA, NP], BF16, kind="ExternalInput").ap()
    d_mask = nc.dram_tensor("maskT", [BL, NP, NP], BF16, kind="ExternalInput").ap()
    d_rpb = nc.dram_tensor("rpbT", [NP, H * NP], BF16, kind="ExternalInput").ap()
    d_wq = nc.dram_tensor("wq", [CA, C], BF16, kind="ExternalInput").ap()
    d_wk = nc.dram_tensor("wk", [CA, C], BF16, kind="ExternalInput").ap()
    d_wv = nc.dram_tensor("wv", [CA, C], BF16, kind="ExternalInput").ap()
    d_wp = nc.dram_tensor("wp", [CA, C], F32R, kind="ExternalInput").ap()
    d_id = nc.dram_tensor("ident", [128, 128], BF16, kind="ExternalInput").ap()
    d_ones = nc.dram_tensor("onesc", [128, 2], BF16, kind="ExternalInput").ap()
    d_onesb = nc.dram_tensor("onesb", [1, 32], BF16, kind="ExternalInput").ap()
    d_tail = nc.dram_tensor("tail", [2, NP], F32R, kind="ExternalInput").ap()
    d_out = nc.dram_tensor("out", [BL, N, C], F32, kind="ExternalOutput").ap()

    with tile.TileContext(nc) as tc, ExitStack() as ctx:
        cpool = ctx.enter_context(tc.tile_pool(name="const", bufs=1))
        xpool = ctx.enter_context(tc.tile_pool(name="x", bufs=2))
        qkpool = ctx.enter_context(tc.tile_pool(name="qk", bufs=2))
        vpool = ctx.enter_context(tc.tile_pool(name="v", bufs=2))
        mpool = ctx.enter_context(tc.tile_pool(name="maskt", bufs=2))
        epool = ctx.enter_context(tc.tile_pool(name="e", bufs=6))
        evpool = ctx.enter_context(tc.tile_pool(name="ev", bufs=3))
        opool = ctx.enter_context(tc.tile_pool(name="o", bufs=3))
        rpool = ctx.enter_context(tc.tile_pool(name="r", bufs=3))
        fpool = ctx.enter_context(tc.tile_pool(name="fin", bufs=3))
        ps_a = ctx.enter_context(tc.tile_pool(name="ps_a", bufs=1, space="PSUM"))
        ps_pv = ctx.enter_context(tc.tile_pool(name="ps_pv", bufs=1, space="PSUM"))
        ps_s = ctx.enter_context(tc.tile_pool(name="ps_s", bufs=1, space="PSUM"))
        ps_m = ctx.enter_context(tc.tile_pool(name="ps_m", bufs=2, space="PSUM"))

        # ---- resident constants ----
        wq_t, wk_t, wv_t, wp_t = [], [], [], []
        for ki, (ko, kn) in enumerate(KCH):
            for lst, src, dty in ((wq_t, d_wq, BF16), (wk_t, d_wk, BF16),
                                  (wv_t, d_wv, BF16), (wp_t, d_wp, F32R)):
                t = cpool.tile([kn, C], dty, tag=f"w{len(lst)}_{ki}_{id(lst) % 97}")
                nc.sync.dma_start(t[:], src[ko:ko + kn, :])
                lst.append(t)
        id_t = cpool.tile([128, 128], BF16, tag="ident")
        nc.sync.dma_start(id_t[:], d_id[:, :])
        ones_t = cpool.tile([128, 2], BF16, tag="onesc")
        nc.sync.dma_start(ones_t[:], d_ones[:, :])
        onesb_t = cpool.tile([1, 32], BF16, tag="onesb")
        nc.sync.dma_start(onesb_t[:], d_onesb[:, :])
        tail_t = cpool.tile([2, NP], F32R, tag="tail")
        nc.sync.dma_start(tail_t[:], d_tail[:, :])
        rpb_t = []
        for ti, (to, tn) in enumerate(TCH):
            t = cpool.tile([tn, H * NP], BF16, tag=f"rpb{ti}")
            nc.sync.dma_start(t[:], d_rpb[to:to + tn, :])
            rpb_t.append(t)

        def emit_head(b, tail_fn=None):
            # ---- load x^T ----
            xq_t, xk_t = [], []
            for ki, (ko, kn) in enumerate(KCH):
                for lst, srcd, nm in ((xq_t, d_xq, "xq"), (xk_t, d_xk, "xk")):
                    t = xpool.tile([kn, NP], BF16, tag=f"{nm}{ki}")
                    nc.sync.dma_start(t[:], srcd[b, ko:ko + kn, :])
                    lst.append(t)
            mask_t = []
            for ti, (to, tn) in enumerate(TCH):
                t = mpool.tile([tn, NP], BF16, tag=f"mask{ti}")
                nc.scalar.dma_start(t[:], d_mask[b, to:to + tn, :])
                mask_t.append(t)

            # ---- Q^T, K^T projections (fp32r) ----
            qt, kt = [], []
            for w_t, x_t, dest, nm in ((wq_t, xq_t, None, "q"), (wk_t, xk_t, None, "k")):
                dest = qt if nm == "q" else kt
                for mi, (mo, mn) in enumerate(MCH):
                    ps = ps_m.tile([128, 512], F32, tag="mm")
                    for ki in range(len(KCH)):
                        nc.tensor.matmul(ps[0:mn, 0:NP],
                                         w_t[ki][:, mo:mo + mn],
                                         x_t[ki][:],
                                         start=(ki == 0), stop=(ki == len(KCH) - 1))
                    sb = qkpool.tile([96, NP], BF16, tag=f"{nm}{mi}")
                    nc.vector.tensor_copy(sb[:], ps[0:mn, 0:NP])
                    dest.append(sb)

            # ---- V projection (bf16, natural layout) ----
            v_t = []
            for ti, (to, tn) in enumerate(TCH):
                ps = ps_m.tile([128, 512], F32, tag="mm")
                for ki in range(len(KCH)):
                    nc.tensor.matmul(ps[0:tn, 0:C],
                                     xk_t[ki][:, to:to + tn],
                                     wv_t[ki][:],
                                     start=(ki == 0), stop=(ki == len(KCH) - 1))
                sb = vpool.tile([128, C], BF16, tag=f"v{ti}")
                nc.vector.tensor_copy(sb[0:tn, :], ps[0:tn, 0:C])
                v_t.append(sb)

            # ---- attention ----
            pv_ps = ps_pv.tile([128, 2, 512], F32, tag="pv")
            s_ps = ps_s.tile([128, 1, 512], F32, tag="s")
            for ci, (co_, cn) in enumerate(TCH):
                e_ts = []
                for g in range(2):
                    a_ps = ps_a.tile([128, 3, 512], F32, tag="a")
                    # QK matmuls first (row-tiled mode, concurrent heads)
                    for hh in range(3):
                        h = 3 * g + hh
                        t_i, r_off = (0, 32 * h) if h < 3 else (1, 32 * (h - 3))
                        nc.tensor.matmul(
                            a_ps[0:cn, hh, 0:NP],
                            kt[t_i][r_off:r_off + D, co_:co_ + cn],
                            qt[t_i][r_off:r_off + D, :],
                            start=True, stop=False)
                    # bias folds (128x128 mode)
                    for hh in range(3):
                        h = 3 * g + hh
                        nc.tensor.matmul(
                            a_ps[0:cn, hh, 0:NP],
                            id_t[0:cn, 0:cn],
                            mask_t[ci][:, :],
                            start=False, stop=False)
                        nc.tensor.matmul(
                            a_ps[0:cn, hh, 0:NP],
                            id_t[0:cn, 0:cn],
                            rpb_t[ci][:, h * NP:(h + 1) * NP],
                            start=False, stop=True)
                    e_t = epool.tile([128, 3 * NP], BF16, tag="e")
                    nc.scalar.activation(
                        e_t[0:cn, :].rearrange("p (r n) -> p r n", r=3),
                        a_ps[0:cn, 0:3, 0:NP],
                        mybir.ActivationFunctionType.Exp)
                    e_ts.append(e_t)

                if ci == 0 and tail_fn is not None:
                    # inject previous batch's tail here: its ACT ops land
                    # early in the ACT FIFO, its proj matmuls after this
                    # chunk's in the PE FIFO.
                    tail_fn()

                for g in range(2):
                    for hh in range(3):
                        h = 3 * g + hh
                        bank, base = (0, 32 * h) if h < 4 else (1, 32 * (h - 4))
                        nc.tensor.matmul(
                            pv_ps[base:base + D, bank, 0:NP],
                            v_t[ci][0:cn, 32 * h:32 * h + D],
                            e_ts[g][0:cn, hh * NP:(hh + 1) * NP],
                            start=(ci == 0), stop=(ci == len(TCH) - 1),
                            tile_position=(0, base))
                        # s rows: heads 0-3 in s bank at rows 32h;
                        # heads 4,5 tucked into pv bank1 rows 64/96.
                        if h < 4:
                            s_out = s_ps[32 * h:32 * h + 1, 0, 0:NP]
                            s_tp = (0, 32 * h)
                        else:
                            s_out = pv_ps[32 * (h - 2):32 * (h - 2) + 1, 1, 0:NP]
                            s_tp = (0, 32 * (h - 2))
                        nc.tensor.matmul(
                            s_out,
                            ones_t[0:cn, 0:1],
                            e_ts[g][0:cn, hh * NP:(hh + 1) * NP],
                            start=(ci == 0), stop=(ci == len(TCH) - 1),
                            tile_position=s_tp)

            return pv_ps, s_ps

        def emit_evac(handles):
            pv_ps, s_ps = handles
            # ---- evacuate PSUM -> SBUF so next batch can reuse pv/s banks ----
            # ev_so layout: cols 0:NP = s rows h0-3 (junk elsewhere);
            # cols NP:2NP = pv bank1 (h4/h5 O rows 0-63, s4/s5 at rows 64/96)
            o1_sb = evpool.tile([128, NP], F32, tag="o1")
            so_sb = evpool.tile([128, 2 * NP], F32, tag="so")
            nc.vector.tensor_copy(o1_sb[:], pv_ps[0:128, 0, 0:NP])
            nc.vector.tensor_copy(so_sb[:, NP:2 * NP], pv_ps[0:128, 1, 0:NP])
            nc.scalar.copy(so_sb[:, 0:NP], s_ps[0:128, 0, 0:NP])
            return o1_sb, so_sb

        def emit_tail_act(handles):
            o1_sb, so_sb = handles
            # ---- r = exp(-ln(s)) on ACT (shared table set with the E exp;
            # DVE reciprocal is an 8-cycle/elem iterative op) ----
            lnr = rpool.tile([128, 2 * NP], F32, tag="lnr")
            nc.scalar.activation(lnr[:], so_sb[:],
                                 mybir.ActivationFunctionType.Ln)
            r_sb = rpool.tile([128, 2 * NP], BF16, tag="r")
            nc.scalar.activation(r_sb[:], lnr[:],
                                 mybir.ActivationFunctionType.Exp, scale=-1.0)

            # ---- gather the 6 r rows into one partition (2 tiny DMAs),
            # then broadcast via K=1 outer-product matmuls into PSUM ----
            rmv = rpool.tile([1, 6 * NP], BF16, tag="rmv")
            rowlen = r_sb[:].tensor.shape[-1]
            base_off = r_sb[:].offset
            src = bass.AP(tensor=r_sb[:].tensor, offset=base_off,
                          ap=[[32 * rowlen, 4], [1, NP]])
            nc.scalar.dma_start(rmv[0:1, 0:4 * NP], src)
            src = bass.AP(tensor=r_sb[:].tensor,
                          offset=base_off + 64 * rowlen + NP,
                          ap=[[32 * rowlen, 2], [1, NP]])
            nc.scalar.dma_start(rmv[0:1, 4 * NP:6 * NP], src)

            rb1 = ps_m.tile([128, 512], F32, tag="mm")
            rb2 = ps_m.tile([128, 512], F32, tag="mm")
            for h in range(4):
                nc.tensor.matmul(rb1[32 * h:32 * h + 32, 0:NP],
                                 onesb_t[0:1, :],
                                 rmv[0:1, h * NP:(h + 1) * NP],
                                 start=True, stop=True, tile_position=(0, 32 * h))
            for h in range(2):
                nc.tensor.matmul(rb2[32 * h:32 * h + 32, 0:NP],
                                 onesb_t[0:1, :],
                                 rmv[0:1, (4 + h) * NP:(5 + h) * NP],
                                 start=True, stop=True, tile_position=(0, 32 * h))
            return rb1, rb2

        def emit_tail_pe(b, handles, rbs):
            o1_sb, so_sb = handles
            rb1, rb2 = rbs
            # ---- normalize O^T -> head-stacked Ostack (fp32r) ----
            ost1 = opool.tile([128, NP], F32R, tag="ost1")
            ost2 = opool.tile([66, NP], F32R, tag="ost2")
            nc.vector.scalar_tensor_tensor(
                ost1[:], o1_sb[:], 1.0, rb1[0:128, 0:NP],
                mybir.AluOpType.mult, mybir.AluOpType.mult)
            nc.vector.scalar_tensor_tensor(
                ost2[0:64, :], so_sb[0:64, NP:2 * NP], 1.0, rb2[0:64, 0:NP],
                mybir.AluOpType.mult, mybir.AluOpType.mult)
            if b < 3:  # pool has 3 slots; the ones/zero tail rows persist
                nc.sync.dma_start(ost2[64:66, :], tail_t[:])

            # ---- output projection (fp32r) + store ----
            ost = [ost1, ost2]
            for ti, (to, tn) in enumerate(TCH):
                ps = ps_m.tile([128, 512], F32, tag="mm")
                for ki in range(2):
                    nc.tensor.matmul(ps[0:tn, 0:C],
                                     ost[ki][:, to:to + tn],
                                     wp_t[ki][:],
                                     start=(ki == 0), stop=(ki == 1))
                f_sb = fpool.tile([128, C], F32, tag="f")
                nc.vector.tensor_copy(f_sb[0:tn, :], ps[0:tn, 0:C])
                rows = min(tn, N - to)
                nc.scalar.dma_start(d_out[b, to:to + rows, :], f_sb[0:rows, :])

        # software pipeline: batch b's tail ACT-chain is injected inside
        # batch b+1's head (after chunk-0 exps, so ln/exp land early in the
        # ACT FIFO); the stt+proj tail runs after the full head; evacuation
        # last so the DVE FIFO has stt(b-1) before evac(b).
        prev = None
        rb_box = {}
        for b in range(BL):
            if prev is not None:
                pe = prev
                tail_fn = lambda pe=pe: rb_box.__setitem__(0, emit_tail_act(pe))
            else:
                tail_fn = None
            ps_handles = emit_head(b, tail_fn)
            if prev is not None:
                emit_tail_pe(b - 1, prev, rb_box[0])
            prev = emit_evac(ps_handles)
        rbs = emit_tail_act(prev)
        emit_tail_pe(BL - 1, prev, rbs)

    nc.compile()
    return nc


_NC_CACHE = None


def _get_program():
    global _NC_CACHE
    if _NC_CACHE is None:
        _NC_CACHE = build_program()
    return _NC_CACHE


def _prep_inputs(x_q, x_kv, mask, q_w, q_b, kv_w, kv_b, proj_w, proj_b,
                 rpb_table, rpi):
    bf16 = ml_dtypes.bfloat16
    f32 = np.float32

    def aug_w(w, bias, scale=1.0):
        m = np.zeros((CA, C), f32)
        m[:C] = np.asarray(w, f32).T
        m[C] = np.asarray(bias, f32)
        return np.ascontiguousarray(m * scale)

    wq = aug_w(q_w, q_b, SCALE).astype(bf16)
    wk = aug_w(kv_w[:C], kv_b[:C]).astype(bf16)
    wv = aug_w(kv_w[C:], kv_b[C:]).astype(bf16)
    wp = aug_w(proj_w, proj_b)

    def xT_aug(x):
        out = np.zeros((B, CA, NP), f32)
        out[:, :C, :N] = np.asarray(x, f32).transpose(0, 2, 1)
        out[:, C, :N] = 1.0
        return out

    xqT = xT_aug(x_q).astype(bf16)
    xkT = xT_aug(x_kv).astype(bf16)

    maskT = np.full((NW, NP, NP), -100.0, f32)
    maskT[:, :N, :N] = np.asarray(mask, f32).transpose(0, 2, 1)
    maskT = maskT.astype(bf16)

    g = np.asarray(rpb_table, f32)[np.asarray(rpi)]        # [i, j, H]
    rpbT = np.zeros((NP, H, NP), f32)
    rpbT[:N, :, :N] = g.transpose(1, 2, 0)
    rpbT = rpbT.reshape(NP, H * NP).astype(bf16)

    ident = np.eye(128, dtype=f32).astype(bf16)
    onesc = np.ones((128, 2), f32).astype(bf16)
    tail = np.zeros((2, NP), f32)
    tail[0, :N] = 1.0

    in_maps = []
    for cidx in range(NCORES):
        sl = slice(cidx * BL, (cidx + 1) * BL)
        w0 = (cidx * BL) % NW
        in_maps.append({
            "xq": xqT[sl], "xk": xkT[sl],
            "maskT": maskT[w0:w0 + BL], "rpbT": rpbT,
            "wq": wq, "wk": wk, "wv": wv, "wp": wp,
            "ident": ident, "onesc": onesc, "tail": tail,
            "onesb": np.ones((1, 32), np.float32).astype(bf16),
        })
    return in_maps


def kernel(x_q, x_kv, mask, q_w, q_b, kv_w, kv_b, proj_w, proj_b,
           rpb_table, rpi):
    nc = _get_program()
    in_maps = _prep_inputs(x_q, x_kv, mask, q_w, q_b, kv_w, kv_b,
                           proj_w, proj_b, rpb_table, rpi)
    res = run_bass_kernel_spmd(nc, in_maps, core_ids=list(range(NCORES)),
                               trace=False)
    out = np.concatenate([res.results[i]["out"] for i in range(NCORES)], 0)
    return np.ascontiguousarray(out.astype(np.float32))


def run_traced(inputs, trace=True):
    """test-harness entry: returns (output, exec_time_ns, results_obj)."""
    nc = _get_program()
    in_maps = _prep_inputs(**inputs)
    res = run_bass_kernel_spmd(nc, in_maps, core_ids=list(range(NCORES)),
                               trace=trace)
    out = np.concatenate([res.results[i]["out"] for i in range(NCORES)], 0)
    return np.ascontiguousarray(out.astype(np.float32)), res.exec_time_ns, res
